# revision 16
# baseline (speedup 1.0000x reference)
"""Trainium2 Bass kernel for nn_NonLocalBlock (multi-head non-local attention
block with conv/BN/SE tail).

Sharding: 8 cores = 2 batches x 4 query(o)-slices of 1024. Each core computes
full attention (all 4 heads, full key length 4096) for its o-slice, the conv
stack on its slice, and joins the SE squeeze via per-chunk 4-core AllGathers.

Key structure:
 - Head-pair layout: Q/K live as [128, cols] tiles where rows 0:64 = even
   head dims, 64:128 = odd head dims (the natural projection-psum layout).
   QK quad-packs the two heads at tile_position (0,0)/(64,0) - no dup DMAs.
 - K bias dropped: softmax over keys is invariant to per-query constants,
   so (Q+bq)*(K+bk) == (Q+bq)*K modulo softmax.
 - exp split across engines: 5/8 of score tiles on ScalarE (native Exp),
   3/8 on DVE via Schraudolph bit-trick: int16(round(s*23.083+16250.5))
   bitcast to bf16 ~= exp(s/8) within 3.3%; softmax cancels most of it.
 - The attention stream is software-pipelined one unit ahead (QK of unit
   k+1 emitted before exp/PV of unit k) so the strict-FIFO PE queue never
   blocks on the exp engines.
 - softmax normalized AFTER PV: ones-column appended to V^T (M=65) yields
   row sums in psum row 64 of the same matmul.
 - BN folded into weights (host); BN-shift + relu on ScalarE activation;
   h2 activation's accum_out produces the SE squeeze partial for free
   (sq path folded on host: fc = relu((Wse1@W3/N) @ sum(h2) + Wse1@b3+bse1)).
"""
import numpy as np
import ml_dtypes

import concourse.bass as bass
import concourse.tile as tile
from concourse import bacc, mybir
from concourse.bass_utils import run_bass_kernel_spmd

FP32 = mybir.dt.float32
BF16 = mybir.dt.bfloat16
I16 = mybir.dt.int16
I8 = mybir.dt.int8
FP8 = mybir.dt.float8e4
DR = mybir.MatmulPerfMode.DoubleRow
ALU = mybir.AluOpType
ACTF = mybir.ActivationFunctionType

C, CH, N, BS, HEADS, DH = 256, 128, 4096, 2, 4, 64
O = 1024          # per-core o-slice
OC = 512          # o-chunk
NT = N // 128     # 32 i-tiles
EPS = 1e-5

# Schraudolph bf16 exp-from-bits: bits = round(x*0.125 * 128*log2(e) + B)
SCH_A = 0.125 * 128.0 * np.log2(np.e)
SCH_B = 127.0 * 128.0 - 5.5
SCH8_A = 0.125 * 8.0 * np.log2(np.e)
SCH8_B = 7.0 * 8.0 - 0.35
SC_PAT = {0, 2, 4, 6, 9, 11, 13, 14, 15}   # it%16 on ScalarE (9/16)

_CACHE = {}


def _build():
    nc = bacc.Bacc(None, target_bir_lowering=False, debug=False)

    di = {}
    def inp(name, shape, dt):
        di[name] = nc.dram_tensor(name, list(shape), dt, kind="ExternalInput")
        return di[name]

    feat_bf = inp("feat_bf", [C, N], BF16)
    feat_q = inp("feat_q", [C, O], BF16)
    feat_res = inp("feat_res", [C, O], FP32)
    feat_bv4 = inp("feat_bv4", [64, HEADS * O], BF16)
    wq_t = inp("wq_t", [C, C], BF16)
    wk_t = inp("wk_t", [C, C], BF16)
    wv_t = inp("wv_t", [C, C], BF16)
    bq2 = inp("bq2", [128, 2], FP32)
    w1_4 = inp("w1_4", [64, 4 * CH], BF16)     # conv1 lhsT, 4 head-chunks
    w2_t = inp("w2_t", [CH, CH], BF16)
    w3_t = inp("w3_t", [CH, C], BF16)
    bn1_b = inp("bn1_b", [128, 1], FP32)
    bn2_b = inp("bn2_b", [128, 1], FP32)
    b3_2 = inp("b3_2", [128, 2], FP32)
    wsq_t = inp("wsq_t", [CH, 16], BF16)       # (Wse1 @ W3 / N)^T
    bsq = inp("bsq", [16, 1], FP32)            # Wse1 @ b3 + bse1
    wse2_t = inp("wse2_t", [16, C], BF16)
    bse2_2 = inp("bse2_2", [128, 2], FP32)     # negated

    out_d = nc.dram_tensor("out", [C, O], FP32, kind="ExternalOutput")

    with tile.TileContext(nc) as tc:
        with (
            tc.tile_pool(name="const", bufs=1) as cpool,
            tc.tile_pool(name="work", bufs=2) as wpool,
            tc.tile_pool(name="et", bufs=3) as epool,
            tc.tile_pool(name="norm", bufs=2) as npool,
            tc.tile_pool(name="psA", bufs=3, space="PSUM") as psA,
            tc.tile_pool(name="psC", bufs=1, space="PSUM") as psC,
            tc.tile_pool(name="dram", bufs=1, space="DRAM") as dpool,
        ):
            # ---------------- load constants / inputs ----------------
            def load(dram, shape, name):
                t = cpool.tile(list(shape), dram.dtype, tag=name, name=name)
                nc.sync.dma_start(t[:], dram[:])
                return t

            def load2(dram, cols, name):
                t = cpool.tile([128, 2 * cols], dram.dtype, tag=name, name=name)
                for ct in range(2):
                    nc.sync.dma_start(t[:, ct * cols:(ct + 1) * cols],
                                      dram[ct * 128:(ct + 1) * 128, :])
                return t

            # DMA order matters: projection weights first (small, unblock the
            # PE), then feat, then everything needed later.
            sb_wv = load2(wv_t, C, "wv")
            sb_wk = load2(wk_t, C, "wk")
            sb_wq = load2(wq_t, C, "wq")      # [128, 2*256] lhsT ch-tiles
            sb_featbf = cpool.tile([128, 2 * N], BF16, tag="featbf")
            for ct in range(2):
                for q4 in range(4):
                    nc.sync.dma_start(
                        sb_featbf[:, ct * N + q4 * 1024: ct * N + (q4 + 1) * 1024],
                        feat_bf[ct * 128:(ct + 1) * 128,
                                q4 * 1024:(q4 + 1) * 1024])
            sb_featq = load2(feat_q, O, "featq")
            sb_featres = cpool.tile([128, 2 * O], FP32, tag="featres")
            for ct in range(2):
                nc.sync.dma_start(sb_featres[:, ct * O:(ct + 1) * O],
                                  feat_res[ct * 128:(ct + 1) * 128, :])
            sb_featbv4 = load(feat_bv4, [64, HEADS * O], "featbv4")
            sb_w14 = load(w1_4, [64, 4 * CH], "w14")
            sb_w2 = load(w2_t, [128, CH], "w2")
            sb_w3 = load(w3_t, [128, C], "w3")
            sb_wsq = load(wsq_t, [128, 16], "wsq")
            sb_wse2 = load(wse2_t, [16, C], "wse2")
            sb_bq2 = load(bq2, [128, 2], "bq2")
            sb_bn1b = load(bn1_b, [128, 1], "bn1b")
            sb_bn2b = load(bn2_b, [128, 1], "bn2b")
            sb_b32 = load(b3_2, [128, 2], "b32")
            sb_bsq = load(bsq, [16, 1], "bsq")
            sb_bse22 = load(bse2_2, [128, 2], "bse22")
            sb_ones = cpool.tile([65, 64], FP32, tag="ones")
            nc.vector.memset(sb_ones[:], 1.0)

            # ---------------- V^T (fp8, DoubleRow layout) ----------------
            # vt2 block (itp, j, h): cols itp*640 + j*320 + h*80 + [0:65];
            # cols 0-63 = V rows for key-tile 2*itp+j, col 64 stays 1.0 so
            # the DoubleRow PV matmul (M=65, K=256) emits row sums in psum
            # row 64.  80-col pitch keeps the j-stride 16B-aligned.
            vt = cpool.tile([128, (NT // 2) * 640], FP8, tag="vt")
            nc.gpsimd.memset(vt[:], 1.0)
            for itp in range(NT // 2):
                ps = psA.tile([128, 2 * OC], FP32, tag="s", name="vps")
                for half in range(2):
                    it = 2 * itp + half
                    for ch in range(2):
                        nc.tensor.matmul(
                            ps[:, half * 256:half * 256 + 256],
                            sb_featbf[:, ch * N + it * 128: ch * N + it * 128 + 128],
                            sb_wv[:, ch * C:(ch + 1) * C],
                            start=(ch == 0), stop=(ch == 1),
                            skip_group_check=True)
                dst = vt[:, itp * 640:(itp + 1) * 640] \
                    .rearrange("p (j h z) -> p j h z", j=2, h=4)[:, :, :, 0:64]
                src = ps[:, 0:512].rearrange("p (j h k) -> p j h k", j=2, k=64)
                if itp % 2 == 0:
                    nc.vector.tensor_copy(dst, src)
                else:
                    nc.scalar.activation(dst, src, ACTF.Copy)

            # ---------------- K / Q projections (head-pair layout) --------
            # kp ct-block: [128, N] rows 0:64 = head 2ct dims, 64:128 = 2ct+1
            kp = cpool.tile([128, 2 * N], BF16, tag="kp")
            qp = cpool.tile([128, 2 * O], BF16, tag="qp")
            for ct in range(2):
                for oc4 in range(4):
                    psk = psA.tile([128, 2 * OC], FP32, tag="s", name="psk")
                    for half in range(2):
                        for ch in range(2):
                            nc.tensor.matmul(
                                psk[:, half * OC:(half + 1) * OC],
                                sb_wk[:, ch * C + ct * 128: ch * C + (ct + 1) * 128],
                                sb_featbf[:, ch * N + oc4 * 1024 + half * OC:
                                          ch * N + oc4 * 1024 + (half + 1) * OC],
                                start=(ch == 0), stop=(ch == 1))
                    kslice = kp[:, ct * N + oc4 * 1024: ct * N + (oc4 + 1) * 1024]
                    if ct == 0:
                        nc.vector.tensor_copy(kslice, psk[:])
                    else:
                        nc.scalar.activation(kslice, psk[:], ACTF.Copy)
                psq = psA.tile([128, 2 * OC], FP32, tag="s", name="psq")
                for half in range(2):
                    for ch in range(2):
                        nc.tensor.matmul(
                            psq[:, half * OC:(half + 1) * OC],
                            sb_wq[:, ch * C + ct * 128: ch * C + (ct + 1) * 128],
                            sb_featq[:, ch * O + half * OC:
                                     ch * O + (half + 1) * OC],
                            start=(ch == 0), stop=(ch == 1))
                nc.vector.tensor_scalar_add(
                    qp[:, ct * O:(ct + 1) * O], psq[:], sb_bq2[:, ct:ct + 1])

            # ---------------- attention (software-pipelined) ----------------
            msg_sb = cpool.tile([128, 2 * O], FP32, tag="msg")
            pvs = {}      # (oc, ct) -> pv psum tile
            x4s = {}      # oc -> conv1 input [64, 4*OC]
            cc_outs = {}

            def emit_qk(oc, ct, it):
                oco = oc * OC
                sps = psA.tile([128, 2 * OC], FP32, tag="s", name="sps")
                nc.tensor.matmul(
                    sps[:, 0:OC],
                    kp[0:64, ct * N + it * 128: ct * N + (it + 1) * 128],
                    qp[0:64, ct * O + oco: ct * O + oco + OC],
                    start=True, stop=True, tile_position=(0, 0))
                nc.tensor.matmul(
                    sps[:, OC:2 * OC],
                    kp[64:128, ct * N + it * 128: ct * N + (it + 1) * 128],
                    qp[64:128, ct * O + oco: ct * O + oco + OC],
                    start=True, stop=True, tile_position=(64, 0))
                return sps

            uctr = [0]
            et2_cur = {}

            def emit_expv(oc, ct, it, sps):
                if (oc, ct) not in pvs:
                    pvs[(oc, ct)] = psC.tile([128, 2 * OC], FP32, tag="pv",
                                             name=f"pv{oc}{ct}")
                pvp = pvs[(oc, ct)]
                uctr[0] += 1
                j = it % 2
                if j == 0:
                    et2_cur[(oc, ct)] = epool.tile([128, 4 * OC], FP8,
                                                   tag="et8", name="et8")
                et2 = et2_cur[(oc, ct)]
                half = et2[:, j * 2 * OC:(j + 1) * 2 * OC]
                # first 12 units ScalarE-only: DVE is still draining the
                # projection/V^T eviction queue at kernel start.
                if uctr[0] <= 12 or it % 16 in SC_PAT:
                    nc.scalar.activation(half, sps[:], ACTF.Exp, scale=0.125)
                else:
                    nc.vector.tensor_scalar(
                        half.bitcast(I8), sps[:], SCH8_A, SCH8_B,
                        ALU.mult, ALU.add)
                if j == 1:
                    itp = it // 2
                    for h in range(2):
                        lhsT = vt[:, itp * 640:(itp + 1) * 640] \
                            .rearrange("p (jj z) -> p jj z", jj=2)[
                                :, :, (2 * ct + h) * 80:(2 * ct + h) * 80 + 65]
                        rhs = et2[:].rearrange("p (jj x) -> p jj x", jj=2)[
                            :, :, h * OC:(h + 1) * OC]
                        nc.tensor.matmul(
                            pvp[0:65, h * OC:(h + 1) * OC], lhsT, rhs,
                            start=(itp == 0), stop=(itp == NT // 2 - 1),
                            perf_mode=DR)
                if it == NT - 1:
                    norm_pair(oc, ct)

            def norm_pair(oc, ct):
                """Evict pv to SBUF (frees the psum ring for the next pair),
                reciprocal of the sums row in place (partition 64), then
                broadcast r across partitions 0:64 with a K=1 PE matmul at
                tile_position (64,0) - no DMA, no GPSIMD on this path (DMAs
                here would queue behind in-flight collectives)."""
                pvp = pvs[(oc, ct)]
                pvc = npool.tile([65, 2 * OC], FP32, tag="pvc", name="pvc")
                if ct == 0:
                    nc.scalar.activation(pvc[:], pvp[0:65, :], ACTF.Copy)
                else:
                    nc.vector.tensor_copy(pvc[:], pvp[0:65, :])
                sbb = psC.tile([128, 2 * OC], FP32, tag="pv", name="sbb")
                for half in range(2):
                    nc.tensor.matmul(
                        sbb[0:64, half * OC:(half + 1) * OC],
                        sb_ones[64:65, :],
                        pvc[64:65, half * OC:(half + 1) * OC],
                        start=True, stop=True, tile_position=(64, 0),
                        skip_group_check=True)
                rb = npool.tile([64, 2 * OC], FP32, tag="rb", name="rb")
                nc.vector.reciprocal_approx_fast(rb[:], sbb[0:64, :])
                mr = wpool.tile([64, 2 * OC], BF16, tag="mr", name="mr")
                nc.vector.tensor_tensor(mr[:], rb[:], pvc[0:64, :], ALU.mult)
                if oc not in x4s:
                    x4s[oc] = wpool.tile([64, 4 * OC], BF16, tag="x4",
                                         name=f"x4_{oc}")
                nc.vector.tensor_tensor(
                    x4s[oc][:, ct * 2 * OC:(ct + 1) * 2 * OC]
                    .rearrange("p (h q) -> p h q", h=2),
                    sb_featbv4[:, 2 * ct * O:(2 * ct + 2) * O]
                    .rearrange("p (h o) -> p h o", h=2)[
                        :, :, oc * OC:oc * OC + OC],
                    mr[:].rearrange("p (h q) -> p h q", h=2),
                    ALU.subtract)

            def attn_seq(oc, unit_list, pending):
                for (ct, it) in unit_list:
                    sps = emit_qk(oc, ct, it)
                    if pending is not None:
                        emit_expv(*pending)
                    pending = (oc, ct, it, sps)
                return pending

            def flush(pending):
                if pending is not None:
                    emit_expv(*pending)
                return None

            def conv_chunk(oc):
                oco = oc * OC
                x4 = x4s[oc]
                ps1 = psA.tile([128, 2 * OC], FP32, tag="s", name="ps1")
                for h in range(4):
                    nc.tensor.matmul(
                        ps1[:, 0:OC], sb_w14[:, h * CH:(h + 1) * CH],
                        x4[:, h * OC:(h + 1) * OC],
                        start=(h == 0), stop=(h == 3))
                h1 = wpool.tile([128, OC], BF16, tag="h1", name="h1")
                nc.scalar.activation(h1[:], ps1[:, 0:OC], ACTF.Relu,
                                     bias=sb_bn1b[:, 0:1])
                ps2 = psA.tile([128, 2 * OC], FP32, tag="s", name="ps2")
                nc.tensor.matmul(ps2[:, 0:OC], sb_w2[:], h1[:],
                                 start=True, stop=True)
                h2 = wpool.tile([128, OC], BF16, tag="h2", name="h2")
                h2s = cpool.tile([128, 1], FP32, tag=f"h2s{oc}",
                                 name=f"h2s{oc}")
                nc.scalar.activation(h2[:], ps2[:, 0:OC], ACTF.Relu,
                                     bias=sb_bn2b[:, 0:1], accum_out=h2s[:])
                ps3 = psA.tile([128, 2 * OC], FP32, tag="s", name="ps3")
                for ct in range(2):
                    nc.tensor.matmul(
                        ps3[:, ct * OC:(ct + 1) * OC],
                        sb_w3[:, ct * 128:(ct + 1) * 128], h2[:],
                        start=True, stop=True, skip_group_check=True)
                for ct in range(2):
                    nc.vector.tensor_scalar_add(
                        msg_sb[:, ct * O + oco: ct * O + oco + OC],
                        ps3[:, ct * OC:(ct + 1) * OC], sb_b32[:, ct:ct + 1])
                # SE squeeze partial: AllGather h2 column-sums across 4 cores
                cc_in = dpool.tile([128, 1], FP32, tag=f"cci{oc}",
                                   name=f"cci{oc}")
                cc_out = dpool.tile([512, 1], FP32, tag=f"cco{oc}",
                                    name=f"cco{oc}")
                cc_outs[oc] = cc_out
                nc.sync.dma_start(cc_in[:], h2s[:])
                nc.gpsimd.collective_compute(
                    "AllGather", ALU.bypass,
                    replica_groups=[[0, 1, 2, 3], [4, 5, 6, 7]],
                    ins=[cc_in.opt()], outs=[cc_out.opt()])

            # emission: chunk0 | 6 units of chunk1 (covers norm-0 latency) |
            # conv0+cc | chunk1 rest | conv1+cc | tail.  The first collective
            # posts early and overlaps most of chunk-1 attention.
            units = [(ct, it) for ct in range(2) for it in range(NT)]
            p = attn_seq(0, units, None)
            p = flush(p)
            p = attn_seq(1, units[:6], None)
            p = flush(p)
            conv_chunk(0)
            p = attn_seq(1, units[6:], None)
            p = flush(p)
            conv_chunk(1)

            # ---------------- SE gate tail ----------------
            sq_g = wpool.tile([128, 8], FP32, tag="sqg")
            for oc in range(2):
                nc.sync.dma_start(
                    sq_g[:, 4 * oc:4 * oc + 4],
                    cc_outs[oc][:].rearrange("(s p) k -> p (s k)", p=128))
            sq_t = wpool.tile([128, 4], FP32, tag="sqt")
            nc.vector.tensor_tensor(sq_t[:], sq_g[:, 0:4], sq_g[:, 4:8],
                                    ALU.add)
            sq_t2 = wpool.tile([128, 2], FP32, tag="sqt2")
            nc.vector.tensor_tensor(sq_t2[:], sq_t[:, 0:2], sq_t[:, 2:4],
                                    ALU.add)
            hs_bf = wpool.tile([128, 1], BF16, tag="hsbf")
            nc.vector.tensor_tensor(hs_bf[:], sq_t2[:, 0:1], sq_t2[:, 1:2],
                                    ALU.add)

            fc_ps = psA.tile([128, 2 * OC], FP32, tag="s", name="fc_ps")
            nc.tensor.matmul(fc_ps[0:16, 0:1], sb_wsq[:, 0:16], hs_bf[:],
                             start=True, stop=True)
            fc_sb = wpool.tile([16, 1], BF16, tag="fc")
            nc.vector.tensor_scalar(fc_sb[:], fc_ps[0:16, 0:1],
                                    sb_bsq[:, 0:1], 0.0, ALU.add, ALU.max)

            g_ps = psC.tile([128, 2 * OC], FP32, tag="pv", name="g_ps")
            for ct in range(2):
                nc.tensor.matmul(g_ps[:, ct:ct + 1],
                                 sb_wse2[:, ct * 128:(ct + 1) * 128],
                                 fc_sb[:], start=True, stop=True,
                                 skip_group_check=True)
            # sigmoid(x) = 1/(1+exp(-x)); bse2 negated on host
            ge = wpool.tile([128, 2], FP32, tag="ge")
            for ct in range(2):
                nc.scalar.activation(ge[:, ct:ct + 1], g_ps[:, ct:ct + 1],
                                     ACTF.Exp, bias=sb_bse22[:, ct:ct + 1],
                                     scale=-1.0)
            nc.vector.tensor_scalar_add(ge[:], ge[:], 1.0)
            gate = wpool.tile([128, 2], FP32, tag="gate")
            nc.vector.reciprocal_approx_fast(gate[:], ge[:])

            # out = feat_res + msg * gate
            for ct in range(2):
                nc.vector.scalar_tensor_tensor(
                    out=msg_sb[:, ct * O:(ct + 1) * O],
                    in0=msg_sb[:, ct * O:(ct + 1) * O],
                    scalar=gate[:, ct:ct + 1],
                    in1=sb_featres[:, ct * O:(ct + 1) * O],
                    op0=ALU.mult, op1=ALU.add)
                nc.sync.dma_start(out_d[ct * 128:(ct + 1) * 128, :],
                                  msg_sb[:, ct * O:(ct + 1) * O])

    nc.compile()
    return nc


def _prep_inputs(inputs):
    bf = ml_dtypes.bfloat16
    f = lambda x: np.ascontiguousarray(np.asarray(x, dtype=np.float32))
    feat = f(inputs["feat"])
    Wq, Wk, Wv = f(inputs["Wq"]), f(inputs["Wk"]), f(inputs["Wv"])
    bq, bv = f(inputs["bq"]), f(inputs["bv"])
    W1, W2, W3 = f(inputs["W1"]), f(inputs["W2"]), f(inputs["W3"])
    b1, b2, b3 = f(inputs["b1"]), f(inputs["b2"]), f(inputs["b3"])
    g1, be1, m1, v1 = f(inputs["g1"]), f(inputs["be1"]), f(inputs["m1"]), f(inputs["v1"])
    g2, be2, m2, v2 = f(inputs["g2"]), f(inputs["be2"]), f(inputs["m2"]), f(inputs["v2"])
    Wse1, Wse2 = f(inputs["Wse1"]), f(inputs["Wse2"])
    bse1, bse2 = f(inputs["bse1"]), f(inputs["bse2"])

    s1 = g1 / np.sqrt(v1 + EPS)
    sh1 = be1 - m1 * s1 + b1 * s1
    W1p = W1 * s1[:, None]
    s2 = g2 / np.sqrt(v2 + EPS)
    sh2 = be2 - m2 * s2 + b2 * s2
    W2p = W2 * s2[:, None]

    w1_4 = np.concatenate(
        [np.ascontiguousarray(W1p[:, 64 * h:64 * h + 64].T) for h in range(4)],
        axis=1)                                            # [64, 4*128]
    wsq = (Wse1 @ W3) / np.float32(N)                      # [16, 128]
    bsqv = Wse1 @ b3 + bse1                                # [16]

    common = {
        "wq_t": np.ascontiguousarray(Wq.T).astype(bf),
        "wk_t": np.ascontiguousarray(Wk.T).astype(bf),
        "wv_t": np.ascontiguousarray(Wv.T).astype(bf),
        "bq2": np.ascontiguousarray(bq.reshape(2, 128).T),
        "w1_4": np.ascontiguousarray(w1_4).astype(bf),
        "w2_t": np.ascontiguousarray(W2p.T).astype(bf),
        "w3_t": np.ascontiguousarray(W3.T).astype(bf),
        "bn1_b": sh1.reshape(128, 1),
        "bn2_b": sh2.reshape(128, 1),
        "b3_2": np.ascontiguousarray(b3.reshape(2, 128).T),
        "wsq_t": np.ascontiguousarray(wsq.T).astype(bf),
        "bsq": bsqv.reshape(16, 1),
        "wse2_t": np.ascontiguousarray(Wse2.T).astype(bf),
        "bse2_2": np.ascontiguousarray((-bse2).reshape(2, 128).T),
    }

    in_maps = []
    for core in range(8):
        b, osl = core // 4, core % 4
        o0 = osl * O
        fb = feat[b]
        m = dict(common)
        m["feat_bf"] = fb.astype(bf)
        m["feat_q"] = np.ascontiguousarray(fb[:, o0:o0 + O]).astype(bf)
        m["feat_res"] = np.ascontiguousarray(fb[:, o0:o0 + O])
        fbv = fb[:, o0:o0 + O] - bv[:, None]
        m["feat_bv4"] = np.ascontiguousarray(
            np.concatenate([fbv[64 * h:64 * h + 64, :] for h in range(4)],
                           axis=1)).astype(bf)
        in_maps.append(m)
    return in_maps


def kernel(**inputs) -> np.ndarray:
    if "nc" not in _CACHE:
        _CACHE["nc"] = _build()
    nc = _CACHE["nc"]
    in_maps = _prep_inputs(inputs)
    res = run_bass_kernel_spmd(nc, in_maps, core_ids=list(range(8)))
    out = np.zeros((BS, C, N), dtype=np.float32)
    for core in range(8):
        b, osl = core // 4, core % 4
        out[b, :, osl * O:(osl + 1) * O] = res.results[core]["out"]
    return out


if __name__ == "__main__":
    import sys
    sys.path.insert(0, "/root/problem")
    from reference import setup_inputs, reference
    inp = {k: np.asarray(v) for k, v in setup_inputs().items()}
    ref = np.asarray(reference(**inp))
    got = kernel(**inp)
    err = np.abs(got - ref)
    print("absmax err:", err.max(), "ref absmax:", np.abs(ref).max())
    print("Relative error:", err.max() / np.abs(ref).max())


# revision 18
# speedup vs baseline: 1.0435x; 1.0435x over previous
"""Trainium2 Bass kernel for nn_NonLocalBlock (multi-head non-local attention
block with conv/BN/SE tail).

Sharding: 8 cores = 2 batches x 4 query(o)-slices of 1024. Each core computes
full attention (all 4 heads, full key length 4096) for its o-slice, the conv
stack on its slice, and joins the SE squeeze via per-chunk 4-core AllGathers.

Key structure:
 - Head-pair layout: Q/K live as [128, cols] tiles where rows 0:64 = even
   head dims, 64:128 = odd head dims (the natural projection-psum layout).
   QK quad-packs the two heads at tile_position (0,0)/(64,0) - no dup DMAs.
 - K bias dropped: softmax over keys is invariant to per-query constants,
   so (Q+bq)*(K+bk) == (Q+bq)*K modulo softmax.
 - exp split across engines: 5/8 of score tiles on ScalarE (native Exp),
   3/8 on DVE via Schraudolph bit-trick: int16(round(s*23.083+16250.5))
   bitcast to bf16 ~= exp(s/8) within 3.3%; softmax cancels most of it.
 - The attention stream is software-pipelined one unit ahead (QK of unit
   k+1 emitted before exp/PV of unit k) so the strict-FIFO PE queue never
   blocks on the exp engines.
 - softmax normalized AFTER PV: ones-column appended to V^T (M=65) yields
   row sums in psum row 64 of the same matmul.
 - BN folded into weights (host); BN-shift + relu on ScalarE activation;
   h2 activation's accum_out produces the SE squeeze partial for free
   (sq path folded on host: fc = relu((Wse1@W3/N) @ sum(h2) + Wse1@b3+bse1)).
"""
import numpy as np
import ml_dtypes

import concourse.bass as bass
import concourse.tile as tile
from concourse import bacc, mybir
from concourse.bass_utils import run_bass_kernel_spmd

FP32 = mybir.dt.float32
BF16 = mybir.dt.bfloat16
I16 = mybir.dt.int16
I8 = mybir.dt.int8
FP8 = mybir.dt.float8e4
DR = mybir.MatmulPerfMode.DoubleRow
ALU = mybir.AluOpType
ACTF = mybir.ActivationFunctionType

C, CH, N, BS, HEADS, DH = 256, 128, 4096, 2, 4, 64
O = 1024          # per-core o-slice
OC = 512          # o-chunk
NT = N // 128     # 32 i-tiles
EPS = 1e-5

# Schraudolph bf16 exp-from-bits: bits = round(x*0.125 * 128*log2(e) + B)
SCH_A = 0.125 * 128.0 * np.log2(np.e)
SCH_B = 127.0 * 128.0 - 5.5
SCH8_A = 0.125 * 8.0 * np.log2(np.e)
SCH8_B = 7.0 * 8.0 - 0.35
SC_PAT = {0, 2, 4, 5, 7}   # it%8 values handled by ScalarE (5/8)

_CACHE = {}


def _build():
    nc = bacc.Bacc(None, target_bir_lowering=False, debug=False)

    di = {}
    def inp(name, shape, dt):
        di[name] = nc.dram_tensor(name, list(shape), dt, kind="ExternalInput")
        return di[name]

    feat_bf = inp("feat_bf", [C, N], BF16)
    feat_q = inp("feat_q", [C, O], BF16)
    feat_res = inp("feat_res", [C, O], FP32)
    feat_bv4 = inp("feat_bv4", [64, HEADS * O], BF16)
    wq_t = inp("wq_t", [C, C], BF16)
    wk_t = inp("wk_t", [C, C], BF16)
    wv_t = inp("wv_t", [C, C], BF16)
    bq2 = inp("bq2", [128, 2], FP32)
    w1_4 = inp("w1_4", [64, 4 * CH], BF16)     # conv1 lhsT, 4 head-chunks
    w2_t = inp("w2_t", [CH, CH], BF16)
    w3_t = inp("w3_t", [CH, C], BF16)
    bn1_b = inp("bn1_b", [128, 1], FP32)
    bn2_b = inp("bn2_b", [128, 1], FP32)
    b3_2 = inp("b3_2", [128, 2], FP32)
    wsq_t = inp("wsq_t", [CH, 16], BF16)       # (Wse1 @ W3 / N)^T
    bsq = inp("bsq", [16, 1], FP32)            # Wse1 @ b3 + bse1
    wse2_t = inp("wse2_t", [16, C], BF16)
    bse2_2 = inp("bse2_2", [128, 2], FP32)     # negated

    out_d = nc.dram_tensor("out", [C, O], FP32, kind="ExternalOutput")

    with tile.TileContext(nc) as tc:
        with (
            tc.tile_pool(name="const", bufs=1) as cpool,
            tc.tile_pool(name="work", bufs=2) as wpool,
            tc.tile_pool(name="et", bufs=3) as epool,
            tc.tile_pool(name="norm", bufs=2) as npool,
            tc.tile_pool(name="psA", bufs=3, space="PSUM") as psA,
            tc.tile_pool(name="psC", bufs=1, space="PSUM") as psC,
            tc.tile_pool(name="dram", bufs=1, space="DRAM") as dpool,
        ):
            # ---------------- load constants / inputs ----------------
            def load(dram, shape, name):
                t = cpool.tile(list(shape), dram.dtype, tag=name, name=name)
                nc.sync.dma_start(t[:], dram[:])
                return t

            def load2(dram, cols, name):
                t = cpool.tile([128, 2 * cols], dram.dtype, tag=name, name=name)
                for ct in range(2):
                    nc.sync.dma_start(t[:, ct * cols:(ct + 1) * cols],
                                      dram[ct * 128:(ct + 1) * 128, :])
                return t

            # DMA order matters: projection weights first (small, unblock the
            # PE), then feat, then everything needed later.
            sb_wv = load2(wv_t, C, "wv")
            sb_wk = load2(wk_t, C, "wk")
            sb_wq = load2(wq_t, C, "wq")      # [128, 2*256] lhsT ch-tiles
            fbf = {}
            for ct in range(2):
                for q4 in range(4):
                    t = cpool.tile([128, 1024], BF16, tag=f"fbf{ct}{q4}",
                                   name=f"fbf{ct}{q4}")
                    nc.sync.dma_start(
                        t[:], feat_bf[ct * 128:(ct + 1) * 128,
                                      q4 * 1024:(q4 + 1) * 1024])
                    fbf[(ct, q4)] = t

            def featbf_ap(ch, col, width):
                q4, off = col // 1024, col % 1024
                return fbf[(ch, q4)][:, off:off + width]
            sb_featq = load2(feat_q, O, "featq")
            sb_featres = cpool.tile([128, 2 * O], FP32, tag="featres")
            for ct in range(2):
                nc.sync.dma_start(sb_featres[:, ct * O:(ct + 1) * O],
                                  feat_res[ct * 128:(ct + 1) * 128, :])
            sb_featbv4 = load(feat_bv4, [64, HEADS * O], "featbv4")
            sb_w14 = load(w1_4, [64, 4 * CH], "w14")
            sb_w2 = load(w2_t, [128, CH], "w2")
            sb_w3 = load(w3_t, [128, C], "w3")
            sb_wsq = load(wsq_t, [128, 16], "wsq")
            sb_wse2 = load(wse2_t, [16, C], "wse2")
            sb_bq2 = load(bq2, [128, 2], "bq2")
            sb_bn1b = load(bn1_b, [128, 1], "bn1b")
            sb_bn2b = load(bn2_b, [128, 1], "bn2b")
            sb_b32 = load(b3_2, [128, 2], "b32")
            sb_bsq = load(bsq, [16, 1], "bsq")
            sb_bse22 = load(bse2_2, [128, 2], "bse22")
            sb_ones = cpool.tile([65, 64], FP32, tag="ones")
            nc.vector.memset(sb_ones[:], 1.0)

            # ---------------- V^T (fp8, DoubleRow layout) ----------------
            # vt2 block (itp, j, h): cols itp*640 + j*320 + h*80 + [0:65];
            # cols 0-63 = V rows for key-tile 2*itp+j, col 64 stays 1.0 so
            # the DoubleRow PV matmul (M=65, K=256) emits row sums in psum
            # row 64.  80-col pitch keeps the j-stride 16B-aligned.
            vt = cpool.tile([128, (NT // 2) * 640], FP8, tag="vt")
            nc.gpsimd.memset(
                vt[:].rearrange("p (b z) -> p b z", z=80)[:, :, 64:65], 1.0)
            for itp in range(NT // 2):
                ps = psA.tile([128, 2 * OC], FP32, tag="s", name="vps")
                for half in range(2):
                    it = 2 * itp + half
                    for ch in range(2):
                        nc.tensor.matmul(
                            ps[:, half * 256:half * 256 + 256],
                            featbf_ap(ch, it * 128, 128),
                            sb_wv[:, ch * C:(ch + 1) * C],
                            start=(ch == 0), stop=(ch == 1),
                            skip_group_check=True)
                dst = vt[:, itp * 640:(itp + 1) * 640] \
                    .rearrange("p (j h z) -> p j h z", j=2, h=4)[:, :, :, 0:64]
                src = ps[:, 0:512].rearrange("p (j h k) -> p j h k", j=2, k=64)
                if itp % 2 == 0:
                    nc.vector.tensor_copy(dst, src)
                else:
                    nc.scalar.activation(dst, src, ACTF.Copy)

            # ---------------- K / Q projections (head-pair layout) --------
            # kp ct-block: [128, N] rows 0:64 = head 2ct dims, 64:128 = 2ct+1
            kp = cpool.tile([128, 2 * N], BF16, tag="kp")
            qp = cpool.tile([128, 2 * O], BF16, tag="qp")
            for ct in range(2):
                for oc4 in range(4):
                    psk = psA.tile([128, 2 * OC], FP32, tag="s", name="psk")
                    for half in range(2):
                        for ch in range(2):
                            nc.tensor.matmul(
                                psk[:, half * OC:(half + 1) * OC],
                                sb_wk[:, ch * C + ct * 128: ch * C + (ct + 1) * 128],
                                featbf_ap(ch, oc4 * 1024 + half * OC, OC),
                                start=(ch == 0), stop=(ch == 1))
                    kslice = kp[:, ct * N + oc4 * 1024: ct * N + (oc4 + 1) * 1024]
                    if ct == 0:
                        nc.vector.tensor_copy(kslice, psk[:])
                    else:
                        nc.scalar.activation(kslice, psk[:], ACTF.Copy)
                psq = psA.tile([128, 2 * OC], FP32, tag="s", name="psq")
                for half in range(2):
                    for ch in range(2):
                        nc.tensor.matmul(
                            psq[:, half * OC:(half + 1) * OC],
                            sb_wq[:, ch * C + ct * 128: ch * C + (ct + 1) * 128],
                            sb_featq[:, ch * O + half * OC:
                                     ch * O + (half + 1) * OC],
                            start=(ch == 0), stop=(ch == 1))
                nc.vector.tensor_scalar_add(
                    qp[:, ct * O:(ct + 1) * O], psq[:], sb_bq2[:, ct:ct + 1])

            # ---------------- attention (software-pipelined) ----------------
            msg_sb = cpool.tile([128, 2 * O], FP32, tag="msg")
            pvs = {}      # (oc, ct) -> pv psum tile
            x4s = {}      # oc -> conv1 input [64, 4*OC]
            cc_outs = {}

            def emit_qk(oc, ct, it):
                oco = oc * OC
                sps = psA.tile([128, 2 * OC], FP32, tag="s", name="sps")
                nc.tensor.matmul(
                    sps[:, 0:OC],
                    kp[0:64, ct * N + it * 128: ct * N + (it + 1) * 128],
                    qp[0:64, ct * O + oco: ct * O + oco + OC],
                    start=True, stop=True, tile_position=(0, 0))
                nc.tensor.matmul(
                    sps[:, OC:2 * OC],
                    kp[64:128, ct * N + it * 128: ct * N + (it + 1) * 128],
                    qp[64:128, ct * O + oco: ct * O + oco + OC],
                    start=True, stop=True, tile_position=(64, 0))
                return sps

            uctr = [0]
            et2_cur = {}

            def emit_expv(oc, ct, it, sps):
                if (oc, ct) not in pvs:
                    pvs[(oc, ct)] = psC.tile([128, 2 * OC], FP32, tag="pv",
                                             name=f"pv{oc}{ct}")
                pvp = pvs[(oc, ct)]
                uctr[0] += 1
                j = it % 2
                if j == 0:
                    et2_cur[(oc, ct)] = epool.tile([128, 4 * OC], FP8,
                                                   tag="et8", name="et8")
                et2 = et2_cur[(oc, ct)]
                half = et2[:, j * 2 * OC:(j + 1) * 2 * OC]
                # first 12 units ScalarE-only: DVE is still draining the
                # projection/V^T eviction queue at kernel start.
                if uctr[0] <= 12 or it % 8 in SC_PAT:
                    nc.scalar.activation(half, sps[:], ACTF.Exp, scale=0.125)
                else:
                    nc.vector.tensor_scalar(
                        half.bitcast(I8), sps[:], SCH8_A, SCH8_B,
                        ALU.mult, ALU.add)
                if j == 1:
                    itp = it // 2
                    for h in range(2):
                        lhsT = vt[:, itp * 640:(itp + 1) * 640] \
                            .rearrange("p (jj z) -> p jj z", jj=2)[
                                :, :, (2 * ct + h) * 80:(2 * ct + h) * 80 + 65]
                        rhs = et2[:].rearrange("p (jj x) -> p jj x", jj=2)[
                            :, :, h * OC:(h + 1) * OC]
                        nc.tensor.matmul(
                            pvp[0:65, h * OC:(h + 1) * OC], lhsT, rhs,
                            start=(itp == 0), stop=(itp == NT // 2 - 1),
                            perf_mode=DR)
                if it == NT - 1:
                    norm_pair(oc, ct)

            def norm_pair(oc, ct):
                """Evict pv to SBUF (frees the psum ring for the next pair),
                reciprocal of the sums row in place (partition 64), then
                broadcast r across partitions 0:64 with a K=1 PE matmul at
                tile_position (64,0) - no DMA, no GPSIMD on this path (DMAs
                here would queue behind in-flight collectives)."""
                pvp = pvs[(oc, ct)]
                pvc = npool.tile([65, 2 * OC], FP32, tag="pvc", name="pvc")
                if ct == 0:
                    nc.scalar.activation(pvc[:], pvp[0:65, :], ACTF.Copy)
                else:
                    nc.vector.tensor_copy(pvc[:], pvp[0:65, :])
                sbb = psC.tile([128, 2 * OC], FP32, tag="pv", name="sbb")
                for half in range(2):
                    nc.tensor.matmul(
                        sbb[0:64, half * OC:(half + 1) * OC],
                        sb_ones[64:65, :],
                        pvc[64:65, half * OC:(half + 1) * OC],
                        start=True, stop=True, tile_position=(64, 0),
                        skip_group_check=True)
                rb = npool.tile([64, 2 * OC], FP32, tag="rb", name="rb")
                nc.vector.reciprocal_approx_fast(rb[:], sbb[0:64, :])
                mr = wpool.tile([64, 2 * OC], BF16, tag="mr", name="mr")
                nc.vector.tensor_tensor(mr[:], rb[:], pvc[0:64, :], ALU.mult)
                if oc not in x4s:
                    x4s[oc] = wpool.tile([64, 4 * OC], BF16, tag="x4",
                                         name=f"x4_{oc}")
                nc.vector.tensor_tensor(
                    x4s[oc][:, ct * 2 * OC:(ct + 1) * 2 * OC]
                    .rearrange("p (h q) -> p h q", h=2),
                    sb_featbv4[:, 2 * ct * O:(2 * ct + 2) * O]
                    .rearrange("p (h o) -> p h o", h=2)[
                        :, :, oc * OC:oc * OC + OC],
                    mr[:].rearrange("p (h q) -> p h q", h=2),
                    ALU.subtract)

            def attn_seq(oc, unit_list, pending):
                for (ct, it) in unit_list:
                    sps = emit_qk(oc, ct, it)
                    if pending is not None:
                        emit_expv(*pending)
                    pending = (oc, ct, it, sps)
                return pending

            def flush(pending):
                if pending is not None:
                    emit_expv(*pending)
                return None

            def conv_chunk(oc):
                oco = oc * OC
                x4 = x4s[oc]
                ps1 = psA.tile([128, 2 * OC], FP32, tag="s", name="ps1")
                for h in range(4):
                    nc.tensor.matmul(
                        ps1[:, 0:OC], sb_w14[:, h * CH:(h + 1) * CH],
                        x4[:, h * OC:(h + 1) * OC],
                        start=(h == 0), stop=(h == 3))
                h1 = wpool.tile([128, OC], BF16, tag="h1", name="h1")
                nc.scalar.activation(h1[:], ps1[:, 0:OC], ACTF.Relu,
                                     bias=sb_bn1b[:, 0:1])
                ps2 = psA.tile([128, 2 * OC], FP32, tag="s", name="ps2")
                nc.tensor.matmul(ps2[:, 0:OC], sb_w2[:], h1[:],
                                 start=True, stop=True)
                h2 = wpool.tile([128, OC], BF16, tag="h2", name="h2")
                h2s = cpool.tile([128, 1], FP32, tag=f"h2s{oc}",
                                 name=f"h2s{oc}")
                nc.scalar.activation(h2[:], ps2[:, 0:OC], ACTF.Relu,
                                     bias=sb_bn2b[:, 0:1], accum_out=h2s[:])
                ps3 = psA.tile([128, 2 * OC], FP32, tag="s", name="ps3")
                for ct in range(2):
                    nc.tensor.matmul(
                        ps3[:, ct * OC:(ct + 1) * OC],
                        sb_w3[:, ct * 128:(ct + 1) * 128], h2[:],
                        start=True, stop=True, skip_group_check=True)
                for ct in range(2):
                    nc.vector.tensor_scalar_add(
                        msg_sb[:, ct * O + oco: ct * O + oco + OC],
                        ps3[:, ct * OC:(ct + 1) * OC], sb_b32[:, ct:ct + 1])
                # SE squeeze partial: AllGather h2 column-sums across 4 cores
                cc_in = dpool.tile([128, 1], FP32, tag=f"cci{oc}",
                                   name=f"cci{oc}")
                cc_out = dpool.tile([512, 1], FP32, tag=f"cco{oc}",
                                    name=f"cco{oc}")
                cc_outs[oc] = cc_out
                nc.sync.dma_start(cc_in[:], h2s[:])
                nc.gpsimd.collective_compute(
                    "AllGather", ALU.bypass,
                    replica_groups=[[0, 1, 2, 3], [4, 5, 6, 7]],
                    ins=[cc_in.opt()], outs=[cc_out.opt()])

            # emission: chunk0 | 6 units of chunk1 (covers norm-0 latency) |
            # conv0+cc | chunk1 rest | conv1+cc | tail.  The first collective
            # posts early and overlaps most of chunk-1 attention.
            units = [(ct, it) for ct in range(2) for it in range(NT)]
            p = attn_seq(0, units, None)
            p = flush(p)
            p = attn_seq(1, units[:6], None)
            p = flush(p)
            conv_chunk(0)
            p = attn_seq(1, units[6:], None)
            p = flush(p)
            conv_chunk(1)

            # ---------------- SE gate tail ----------------
            sq_g = wpool.tile([128, 8], FP32, tag="sqg")
            for oc in range(2):
                nc.sync.dma_start(
                    sq_g[:, 4 * oc:4 * oc + 4],
                    cc_outs[oc][:].rearrange("(s p) k -> p (s k)", p=128))
            sq_t = wpool.tile([128, 4], FP32, tag="sqt")
            nc.vector.tensor_tensor(sq_t[:], sq_g[:, 0:4], sq_g[:, 4:8],
                                    ALU.add)
            sq_t2 = wpool.tile([128, 2], FP32, tag="sqt2")
            nc.vector.tensor_tensor(sq_t2[:], sq_t[:, 0:2], sq_t[:, 2:4],
                                    ALU.add)
            hs_bf = wpool.tile([128, 1], BF16, tag="hsbf")
            nc.vector.tensor_tensor(hs_bf[:], sq_t2[:, 0:1], sq_t2[:, 1:2],
                                    ALU.add)

            fc_ps = psA.tile([128, 2 * OC], FP32, tag="s", name="fc_ps")
            nc.tensor.matmul(fc_ps[0:16, 0:1], sb_wsq[:, 0:16], hs_bf[:],
                             start=True, stop=True)
            fc_sb = wpool.tile([16, 1], BF16, tag="fc")
            nc.vector.tensor_scalar(fc_sb[:], fc_ps[0:16, 0:1],
                                    sb_bsq[:, 0:1], 0.0, ALU.add, ALU.max)

            g_ps = psC.tile([128, 2 * OC], FP32, tag="pv", name="g_ps")
            for ct in range(2):
                nc.tensor.matmul(g_ps[:, ct:ct + 1],
                                 sb_wse2[:, ct * 128:(ct + 1) * 128],
                                 fc_sb[:], start=True, stop=True,
                                 skip_group_check=True)
            # sigmoid(x) = 1/(1+exp(-x)); bse2 negated on host
            ge = wpool.tile([128, 2], FP32, tag="ge")
            for ct in range(2):
                nc.scalar.activation(ge[:, ct:ct + 1], g_ps[:, ct:ct + 1],
                                     ACTF.Exp, bias=sb_bse22[:, ct:ct + 1],
                                     scale=-1.0)
            nc.vector.tensor_scalar_add(ge[:], ge[:], 1.0)
            gate = wpool.tile([128, 2], FP32, tag="gate")
            nc.vector.reciprocal_approx_fast(gate[:], ge[:])

            # out = feat_res + msg * gate
            for ct in range(2):
                nc.vector.scalar_tensor_tensor(
                    out=msg_sb[:, ct * O:(ct + 1) * O],
                    in0=msg_sb[:, ct * O:(ct + 1) * O],
                    scalar=gate[:, ct:ct + 1],
                    in1=sb_featres[:, ct * O:(ct + 1) * O],
                    op0=ALU.mult, op1=ALU.add)
                nc.sync.dma_start(out_d[ct * 128:(ct + 1) * 128, :],
                                  msg_sb[:, ct * O:(ct + 1) * O])

    nc.compile()
    return nc


def _prep_inputs(inputs):
    bf = ml_dtypes.bfloat16
    f = lambda x: np.ascontiguousarray(np.asarray(x, dtype=np.float32))
    feat = f(inputs["feat"])
    Wq, Wk, Wv = f(inputs["Wq"]), f(inputs["Wk"]), f(inputs["Wv"])
    bq, bv = f(inputs["bq"]), f(inputs["bv"])
    W1, W2, W3 = f(inputs["W1"]), f(inputs["W2"]), f(inputs["W3"])
    b1, b2, b3 = f(inputs["b1"]), f(inputs["b2"]), f(inputs["b3"])
    g1, be1, m1, v1 = f(inputs["g1"]), f(inputs["be1"]), f(inputs["m1"]), f(inputs["v1"])
    g2, be2, m2, v2 = f(inputs["g2"]), f(inputs["be2"]), f(inputs["m2"]), f(inputs["v2"])
    Wse1, Wse2 = f(inputs["Wse1"]), f(inputs["Wse2"])
    bse1, bse2 = f(inputs["bse1"]), f(inputs["bse2"])

    s1 = g1 / np.sqrt(v1 + EPS)
    sh1 = be1 - m1 * s1 + b1 * s1
    W1p = W1 * s1[:, None]
    s2 = g2 / np.sqrt(v2 + EPS)
    sh2 = be2 - m2 * s2 + b2 * s2
    W2p = W2 * s2[:, None]

    w1_4 = np.concatenate(
        [np.ascontiguousarray(W1p[:, 64 * h:64 * h + 64].T) for h in range(4)],
        axis=1)                                            # [64, 4*128]
    wsq = (Wse1 @ W3) / np.float32(N)                      # [16, 128]
    bsqv = Wse1 @ b3 + bse1                                # [16]

    common = {
        "wq_t": np.ascontiguousarray(Wq.T).astype(bf),
        "wk_t": np.ascontiguousarray(Wk.T).astype(bf),
        "wv_t": np.ascontiguousarray(Wv.T).astype(bf),
        "bq2": np.ascontiguousarray(bq.reshape(2, 128).T),
        "w1_4": np.ascontiguousarray(w1_4).astype(bf),
        "w2_t": np.ascontiguousarray(W2p.T).astype(bf),
        "w3_t": np.ascontiguousarray(W3.T).astype(bf),
        "bn1_b": sh1.reshape(128, 1),
        "bn2_b": sh2.reshape(128, 1),
        "b3_2": np.ascontiguousarray(b3.reshape(2, 128).T),
        "wsq_t": np.ascontiguousarray(wsq.T).astype(bf),
        "bsq": bsqv.reshape(16, 1),
        "wse2_t": np.ascontiguousarray(Wse2.T).astype(bf),
        "bse2_2": np.ascontiguousarray((-bse2).reshape(2, 128).T),
    }

    in_maps = []
    for core in range(8):
        b, osl = core // 4, core % 4
        o0 = osl * O
        fb = feat[b]
        m = dict(common)
        m["feat_bf"] = fb.astype(bf)
        m["feat_q"] = np.ascontiguousarray(fb[:, o0:o0 + O]).astype(bf)
        m["feat_res"] = np.ascontiguousarray(fb[:, o0:o0 + O])
        fbv = fb[:, o0:o0 + O] - bv[:, None]
        m["feat_bv4"] = np.ascontiguousarray(
            np.concatenate([fbv[64 * h:64 * h + 64, :] for h in range(4)],
                           axis=1)).astype(bf)
        in_maps.append(m)
    return in_maps


def kernel(**inputs) -> np.ndarray:
    if "nc" not in _CACHE:
        _CACHE["nc"] = _build()
    nc = _CACHE["nc"]
    in_maps = _prep_inputs(inputs)
    res = run_bass_kernel_spmd(nc, in_maps, core_ids=list(range(8)))
    out = np.zeros((BS, C, N), dtype=np.float32)
    for core in range(8):
        b, osl = core // 4, core % 4
        out[b, :, osl * O:(osl + 1) * O] = res.results[core]["out"]
    return out


if __name__ == "__main__":
    import sys
    sys.path.insert(0, "/root/problem")
    from reference import setup_inputs, reference
    inp = {k: np.asarray(v) for k, v in setup_inputs().items()}
    ref = np.asarray(reference(**inp))
    got = kernel(**inp)
    err = np.abs(got - ref)
    print("absmax err:", err.max(), "ref absmax:", np.abs(ref).max())
    print("Relative error:", err.max() / np.abs(ref).max())


# revision 19
# speedup vs baseline: 1.0723x; 1.0276x over previous
"""Trainium2 Bass kernel for nn_NonLocalBlock (multi-head non-local attention
block with conv/BN/SE tail).

Sharding: 8 cores = 2 batches x 4 query(o)-slices of 1024. Each core computes
full attention (all 4 heads, full key length 4096) for its o-slice, the conv
stack on its slice, and joins the SE squeeze via per-chunk 4-core AllGathers.

Key structure:
 - Head-pair layout: Q/K live as [128, cols] tiles where rows 0:64 = even
   head dims, 64:128 = odd head dims (the natural projection-psum layout).
   QK quad-packs the two heads at tile_position (0,0)/(64,0) - no dup DMAs.
 - K bias dropped: softmax over keys is invariant to per-query constants,
   so (Q+bq)*(K+bk) == (Q+bq)*K modulo softmax.
 - exp split across engines: 5/8 of score tiles on ScalarE (native Exp),
   3/8 on DVE via Schraudolph bit-trick: int16(round(s*23.083+16250.5))
   bitcast to bf16 ~= exp(s/8) within 3.3%; softmax cancels most of it.
 - The attention stream is software-pipelined one unit ahead (QK of unit
   k+1 emitted before exp/PV of unit k) so the strict-FIFO PE queue never
   blocks on the exp engines.
 - softmax normalized AFTER PV: ones-column appended to V^T (M=65) yields
   row sums in psum row 64 of the same matmul.
 - BN folded into weights (host); BN-shift + relu on ScalarE activation;
   h2 activation's accum_out produces the SE squeeze partial for free
   (sq path folded on host: fc = relu((Wse1@W3/N) @ sum(h2) + Wse1@b3+bse1)).
"""
import numpy as np
import ml_dtypes

import concourse.bass as bass
import concourse.tile as tile
from concourse import bacc, mybir
from concourse.bass_utils import run_bass_kernel_spmd

FP32 = mybir.dt.float32
BF16 = mybir.dt.bfloat16
I16 = mybir.dt.int16
I8 = mybir.dt.int8
FP8 = mybir.dt.float8e4
DR = mybir.MatmulPerfMode.DoubleRow
ALU = mybir.AluOpType
ACTF = mybir.ActivationFunctionType

C, CH, N, BS, HEADS, DH = 256, 128, 4096, 2, 4, 64
O = 1024          # per-core o-slice
OC = 512          # o-chunk
NT = N // 128     # 32 i-tiles
EPS = 1e-5

# Schraudolph bf16 exp-from-bits: bits = round(x*0.125 * 128*log2(e) + B)
SCH_A = 0.125 * 128.0 * np.log2(np.e)
SCH_B = 127.0 * 128.0 - 5.5
SCH8_A = 0.125 * 8.0 * np.log2(np.e)
SCH8_B = 7.0 * 8.0 - 0.35
SC_PAT = {0, 2, 4, 5, 7}   # it%8 values handled by ScalarE (5/8)

_CACHE = {}


def _build():
    nc = bacc.Bacc(None, target_bir_lowering=False, debug=False)

    di = {}
    def inp(name, shape, dt):
        di[name] = nc.dram_tensor(name, list(shape), dt, kind="ExternalInput")
        return di[name]

    feat_bf = inp("feat_bf", [C, N], BF16)
    feat_q = inp("feat_q", [C, O], BF16)
    feat_res = inp("feat_res", [C, O], FP32)
    feat_bv4 = inp("feat_bv4", [64, HEADS * O], BF16)
    wq_t = inp("wq_t", [C, C], BF16)
    wk_t = inp("wk_t", [C, C], BF16)
    wv_t = inp("wv_t", [C, C], BF16)
    bq2 = inp("bq2", [128, 2], FP32)
    w1_4 = inp("w1_4", [64, 4 * CH], BF16)     # conv1 lhsT, 4 head-chunks
    w2_t = inp("w2_t", [CH, CH], BF16)
    w3_t = inp("w3_t", [CH, C], BF16)
    bn1_b = inp("bn1_b", [128, 1], FP32)
    bn2_b = inp("bn2_b", [128, 1], FP32)
    b3_2 = inp("b3_2", [128, 2], FP32)
    wsq_t = inp("wsq_t", [CH, 16], BF16)       # (Wse1 @ W3 / N)^T
    bsq = inp("bsq", [16, 1], FP32)            # Wse1 @ b3 + bse1
    wse2_t = inp("wse2_t", [16, C], BF16)
    bse2_2 = inp("bse2_2", [128, 2], FP32)     # negated

    out_d = nc.dram_tensor("out", [C, O], FP32, kind="ExternalOutput")

    with tile.TileContext(nc) as tc:
        with (
            tc.tile_pool(name="const", bufs=1) as cpool,
            tc.tile_pool(name="work", bufs=2) as wpool,
            tc.tile_pool(name="et", bufs=3) as epool,
            tc.tile_pool(name="norm", bufs=2) as npool,
            tc.tile_pool(name="psA", bufs=3, space="PSUM") as psA,
            tc.tile_pool(name="psC", bufs=1, space="PSUM") as psC,
            tc.tile_pool(name="dram", bufs=1, space="DRAM") as dpool,
        ):
            # ---------------- load constants / inputs ----------------
            def load(dram, shape, name):
                t = cpool.tile(list(shape), dram.dtype, tag=name, name=name)
                nc.sync.dma_start(t[:], dram[:])
                return t

            def load2(dram, cols, name):
                t = cpool.tile([128, 2 * cols], dram.dtype, tag=name, name=name)
                for ct in range(2):
                    nc.sync.dma_start(t[:, ct * cols:(ct + 1) * cols],
                                      dram[ct * 128:(ct + 1) * 128, :])
                return t

            # DMA order matters: projection weights first (small, unblock the
            # PE), then feat, then everything needed later.
            sb_wv = load2(wv_t, C, "wv")
            fbf = {}
            for q4 in range(4):
                for ct in range(2):
                    t = cpool.tile([128, 1024], BF16, tag=f"fbf{ct}{q4}",
                                   name=f"fbf{ct}{q4}")
                    nc.sync.dma_start(
                        t[:], feat_bf[ct * 128:(ct + 1) * 128,
                                      q4 * 1024:(q4 + 1) * 1024])
                    fbf[(ct, q4)] = t

            def featbf_ap(ch, col, width):
                q4, off = col // 1024, col % 1024
                return fbf[(ch, q4)][:, off:off + width]
            sb_wk = load2(wk_t, C, "wk")
            sb_wq = load2(wq_t, C, "wq")      # [128, 2*256] lhsT ch-tiles
            sb_featq = load2(feat_q, O, "featq")
            sb_featres = cpool.tile([128, 2 * O], FP32, tag="featres")
            for ct in range(2):
                nc.sync.dma_start(sb_featres[:, ct * O:(ct + 1) * O],
                                  feat_res[ct * 128:(ct + 1) * 128, :])
            sb_featbv4 = load(feat_bv4, [64, HEADS * O], "featbv4")
            sb_w14 = load(w1_4, [64, 4 * CH], "w14")
            sb_w2 = load(w2_t, [128, CH], "w2")
            sb_w3 = load(w3_t, [128, C], "w3")
            sb_wsq = load(wsq_t, [128, 16], "wsq")
            sb_wse2 = load(wse2_t, [16, C], "wse2")
            sb_bq2 = load(bq2, [128, 2], "bq2")
            sb_bn1b = load(bn1_b, [128, 1], "bn1b")
            sb_bn2b = load(bn2_b, [128, 1], "bn2b")
            sb_b32 = load(b3_2, [128, 2], "b32")
            sb_bsq = load(bsq, [16, 1], "bsq")
            sb_bse22 = load(bse2_2, [128, 2], "bse22")
            sb_ones = cpool.tile([65, 64], FP32, tag="ones")
            nc.vector.memset(sb_ones[:], 1.0)

            # ---------------- V^T (fp8, DoubleRow layout) ----------------
            # vt2 block (itp, j, h): cols itp*640 + j*320 + h*80 + [0:65];
            # cols 0-63 = V rows for key-tile 2*itp+j, col 64 stays 1.0 so
            # the DoubleRow PV matmul (M=65, K=256) emits row sums in psum
            # row 64.  80-col pitch keeps the j-stride 16B-aligned.
            vt = cpool.tile([128, (NT // 2) * 640], FP8, tag="vt")
            nc.gpsimd.memset(
                vt[:].rearrange("p (b z) -> p b z", z=80)[:, :, 64:65], 1.0)
            for itp in range(NT // 2):
                ps = psA.tile([128, 2 * OC], FP32, tag="s", name="vps")
                for half in range(2):
                    it = 2 * itp + half
                    for ch in range(2):
                        nc.tensor.matmul(
                            ps[:, half * 256:half * 256 + 256],
                            featbf_ap(ch, it * 128, 128),
                            sb_wv[:, ch * C:(ch + 1) * C],
                            start=(ch == 0), stop=(ch == 1),
                            skip_group_check=True)
                dst = vt[:, itp * 640:(itp + 1) * 640] \
                    .rearrange("p (j h z) -> p j h z", j=2, h=4)[:, :, :, 0:64]
                src = ps[:, 0:512].rearrange("p (j h k) -> p j h k", j=2, k=64)
                if itp % 2 == 0:
                    nc.vector.tensor_copy(dst, src)
                else:
                    nc.scalar.activation(dst, src, ACTF.Copy)

            # ---------------- K / Q projections (head-pair layout) --------
            # kp ct-block: [128, N] rows 0:64 = head 2ct dims, 64:128 = 2ct+1
            kp = cpool.tile([128, 2 * N], BF16, tag="kp")
            qp = cpool.tile([128, 2 * O], BF16, tag="qp")
            for ct in range(2):
                for oc4 in range(4):
                    psk = psA.tile([128, 2 * OC], FP32, tag="s", name="psk")
                    for half in range(2):
                        for ch in range(2):
                            nc.tensor.matmul(
                                psk[:, half * OC:(half + 1) * OC],
                                sb_wk[:, ch * C + ct * 128: ch * C + (ct + 1) * 128],
                                featbf_ap(ch, oc4 * 1024 + half * OC, OC),
                                start=(ch == 0), stop=(ch == 1))
                    kslice = kp[:, ct * N + oc4 * 1024: ct * N + (oc4 + 1) * 1024]
                    if ct == 0:
                        nc.vector.tensor_copy(kslice, psk[:])
                    else:
                        nc.scalar.activation(kslice, psk[:], ACTF.Copy)
                psq = psA.tile([128, 2 * OC], FP32, tag="s", name="psq")
                for half in range(2):
                    for ch in range(2):
                        nc.tensor.matmul(
                            psq[:, half * OC:(half + 1) * OC],
                            sb_wq[:, ch * C + ct * 128: ch * C + (ct + 1) * 128],
                            sb_featq[:, ch * O + half * OC:
                                     ch * O + (half + 1) * OC],
                            start=(ch == 0), stop=(ch == 1))
                nc.vector.tensor_scalar_add(
                    qp[:, ct * O:(ct + 1) * O], psq[:], sb_bq2[:, ct:ct + 1])

            # ---------------- attention (software-pipelined) ----------------
            msg_sb = cpool.tile([128, 2 * O], FP32, tag="msg")
            pvs = {}      # (oc, ct) -> pv psum tile
            x4s = {}      # oc -> conv1 input [64, 4*OC]
            cc_outs = {}

            def emit_qk(oc, ct, it):
                oco = oc * OC
                sps = psA.tile([128, 2 * OC], FP32, tag="s", name="sps")
                nc.tensor.matmul(
                    sps[:, 0:OC],
                    kp[0:64, ct * N + it * 128: ct * N + (it + 1) * 128],
                    qp[0:64, ct * O + oco: ct * O + oco + OC],
                    start=True, stop=True, tile_position=(0, 0))
                nc.tensor.matmul(
                    sps[:, OC:2 * OC],
                    kp[64:128, ct * N + it * 128: ct * N + (it + 1) * 128],
                    qp[64:128, ct * O + oco: ct * O + oco + OC],
                    start=True, stop=True, tile_position=(64, 0))
                return sps

            uctr = [0]
            et2_cur = {}

            def emit_expv(oc, ct, it, sps):
                if (oc, ct) not in pvs:
                    pvs[(oc, ct)] = psC.tile([128, 2 * OC], FP32, tag="pv",
                                             name=f"pv{oc}{ct}")
                pvp = pvs[(oc, ct)]
                uctr[0] += 1
                j = it % 2
                if j == 0:
                    et2_cur[(oc, ct)] = epool.tile([128, 4 * OC], FP8,
                                                   tag="et8", name="et8")
                et2 = et2_cur[(oc, ct)]
                half = et2[:, j * 2 * OC:(j + 1) * 2 * OC]
                # first 12 units ScalarE-only: DVE is still draining the
                # projection/V^T eviction queue at kernel start.
                if uctr[0] <= 12 or it % 8 in SC_PAT:
                    nc.scalar.activation(half, sps[:], ACTF.Exp, scale=0.125)
                else:
                    nc.vector.tensor_scalar(
                        half.bitcast(I8), sps[:], SCH8_A, SCH8_B,
                        ALU.mult, ALU.add)
                if j == 1:
                    itp = it // 2
                    for h in range(2):
                        lhsT = vt[:, itp * 640:(itp + 1) * 640] \
                            .rearrange("p (jj z) -> p jj z", jj=2)[
                                :, :, (2 * ct + h) * 80:(2 * ct + h) * 80 + 65]
                        rhs = et2[:].rearrange("p (jj x) -> p jj x", jj=2)[
                            :, :, h * OC:(h + 1) * OC]
                        nc.tensor.matmul(
                            pvp[0:65, h * OC:(h + 1) * OC], lhsT, rhs,
                            start=(itp == 0), stop=(itp == NT // 2 - 1),
                            perf_mode=DR)
                if it == NT - 1:
                    norm_pair(oc, ct)

            def norm_pair(oc, ct):
                """Evict pv to SBUF (frees the psum ring for the next pair),
                reciprocal of the sums row in place (partition 64), then
                broadcast r across partitions 0:64 with a K=1 PE matmul at
                tile_position (64,0) - no DMA, no GPSIMD on this path (DMAs
                here would queue behind in-flight collectives)."""
                pvp = pvs[(oc, ct)]
                pvc = npool.tile([65, 2 * OC], FP32, tag="pvc", name="pvc")
                if ct == 0:
                    nc.scalar.activation(pvc[:], pvp[0:65, :], ACTF.Copy)
                else:
                    nc.vector.tensor_copy(pvc[:], pvp[0:65, :])
                sbb = psC.tile([128, 2 * OC], FP32, tag="pv", name="sbb")
                for half in range(2):
                    nc.tensor.matmul(
                        sbb[0:64, half * OC:(half + 1) * OC],
                        sb_ones[64:65, :],
                        pvc[64:65, half * OC:(half + 1) * OC],
                        start=True, stop=True, tile_position=(64, 0),
                        skip_group_check=True)
                rb = npool.tile([64, 2 * OC], FP32, tag="rb", name="rb")
                nc.vector.reciprocal_approx_fast(rb[:], sbb[0:64, :])
                mr = wpool.tile([64, 2 * OC], BF16, tag="mr", name="mr")
                nc.vector.tensor_tensor(mr[:], rb[:], pvc[0:64, :], ALU.mult)
                if oc not in x4s:
                    x4s[oc] = wpool.tile([64, 4 * OC], BF16, tag="x4",
                                         name=f"x4_{oc}")
                nc.vector.tensor_tensor(
                    x4s[oc][:, ct * 2 * OC:(ct + 1) * 2 * OC]
                    .rearrange("p (h q) -> p h q", h=2),
                    sb_featbv4[:, 2 * ct * O:(2 * ct + 2) * O]
                    .rearrange("p (h o) -> p h o", h=2)[
                        :, :, oc * OC:oc * OC + OC],
                    mr[:].rearrange("p (h q) -> p h q", h=2),
                    ALU.subtract)

            def attn_seq(oc, unit_list, pending):
                for (ct, it) in unit_list:
                    sps = emit_qk(oc, ct, it)
                    if pending is not None:
                        emit_expv(*pending)
                    pending = (oc, ct, it, sps)
                return pending

            def flush(pending):
                if pending is not None:
                    emit_expv(*pending)
                return None

            def conv_chunk(oc):
                oco = oc * OC
                x4 = x4s[oc]
                ps1 = psA.tile([128, 2 * OC], FP32, tag="s", name="ps1")
                for h in range(4):
                    nc.tensor.matmul(
                        ps1[:, 0:OC], sb_w14[:, h * CH:(h + 1) * CH],
                        x4[:, h * OC:(h + 1) * OC],
                        start=(h == 0), stop=(h == 3))
                h1 = wpool.tile([128, OC], BF16, tag="h1", name="h1")
                nc.scalar.activation(h1[:], ps1[:, 0:OC], ACTF.Relu,
                                     bias=sb_bn1b[:, 0:1])
                ps2 = psA.tile([128, 2 * OC], FP32, tag="s", name="ps2")
                nc.tensor.matmul(ps2[:, 0:OC], sb_w2[:], h1[:],
                                 start=True, stop=True)
                h2 = wpool.tile([128, OC], BF16, tag="h2", name="h2")
                h2s = cpool.tile([128, 1], FP32, tag=f"h2s{oc}",
                                 name=f"h2s{oc}")
                nc.scalar.activation(h2[:], ps2[:, 0:OC], ACTF.Relu,
                                     bias=sb_bn2b[:, 0:1], accum_out=h2s[:])
                # SE squeeze partial: AllGather h2 column-sums across 4
                # cores - post before conv3 so the collective starts ASAP
                cc_in = dpool.tile([128, 1], FP32, tag=f"cci{oc}",
                                   name=f"cci{oc}")
                cc_out = dpool.tile([512, 1], FP32, tag=f"cco{oc}",
                                    name=f"cco{oc}")
                cc_outs[oc] = cc_out
                nc.sync.dma_start(cc_in[:], h2s[:])
                nc.gpsimd.collective_compute(
                    "AllGather", ALU.bypass,
                    replica_groups=[[0, 1, 2, 3], [4, 5, 6, 7]],
                    ins=[cc_in.opt()], outs=[cc_out.opt()])
                ps3 = psA.tile([128, 2 * OC], FP32, tag="s", name="ps3")
                for ct in range(2):
                    nc.tensor.matmul(
                        ps3[:, ct * OC:(ct + 1) * OC],
                        sb_w3[:, ct * 128:(ct + 1) * 128], h2[:],
                        start=True, stop=True, skip_group_check=True)
                for ct in range(2):
                    nc.vector.tensor_scalar_add(
                        msg_sb[:, ct * O + oco: ct * O + oco + OC],
                        ps3[:, ct * OC:(ct + 1) * OC], sb_b32[:, ct:ct + 1])

            # emission: chunk0 | 6 units of chunk1 (covers norm-0 latency) |
            # conv0+cc | chunk1 rest | conv1+cc | tail.  The first collective
            # posts early and overlaps most of chunk-1 attention.
            units = [(ct, it) for ct in range(2) for it in range(NT)]
            p = attn_seq(0, units, None)
            p = flush(p)
            p = attn_seq(1, units[:6], None)
            p = flush(p)
            conv_chunk(0)
            p = attn_seq(1, units[6:], None)
            p = flush(p)
            conv_chunk(1)

            # ---------------- SE gate tail ----------------
            sq_g = wpool.tile([128, 8], FP32, tag="sqg")
            for oc in range(2):
                nc.sync.dma_start(
                    sq_g[:, 4 * oc:4 * oc + 4],
                    cc_outs[oc][:].rearrange("(s p) k -> p (s k)", p=128))
            sq_t = wpool.tile([128, 4], FP32, tag="sqt")
            nc.vector.tensor_tensor(sq_t[:], sq_g[:, 0:4], sq_g[:, 4:8],
                                    ALU.add)
            sq_t2 = wpool.tile([128, 2], FP32, tag="sqt2")
            nc.vector.tensor_tensor(sq_t2[:], sq_t[:, 0:2], sq_t[:, 2:4],
                                    ALU.add)
            hs_bf = wpool.tile([128, 1], BF16, tag="hsbf")
            nc.vector.tensor_tensor(hs_bf[:], sq_t2[:, 0:1], sq_t2[:, 1:2],
                                    ALU.add)

            fc_ps = psA.tile([128, 2 * OC], FP32, tag="s", name="fc_ps")
            nc.tensor.matmul(fc_ps[0:16, 0:1], sb_wsq[:, 0:16], hs_bf[:],
                             start=True, stop=True)
            fc_sb = wpool.tile([16, 1], BF16, tag="fc")
            nc.vector.tensor_scalar(fc_sb[:], fc_ps[0:16, 0:1],
                                    sb_bsq[:, 0:1], 0.0, ALU.add, ALU.max)

            g_ps = psC.tile([128, 2 * OC], FP32, tag="pv", name="g_ps")
            for ct in range(2):
                nc.tensor.matmul(g_ps[:, ct:ct + 1],
                                 sb_wse2[:, ct * 128:(ct + 1) * 128],
                                 fc_sb[:], start=True, stop=True,
                                 skip_group_check=True)
            # sigmoid(x) = 1/(1+exp(-x)); bse2 negated on host
            ge = wpool.tile([128, 2], FP32, tag="ge")
            for ct in range(2):
                nc.scalar.activation(ge[:, ct:ct + 1], g_ps[:, ct:ct + 1],
                                     ACTF.Exp, bias=sb_bse22[:, ct:ct + 1],
                                     scale=-1.0)
            nc.vector.tensor_scalar_add(ge[:], ge[:], 1.0)
            gate = wpool.tile([128, 2], FP32, tag="gate")
            nc.vector.reciprocal_approx_fast(gate[:], ge[:])

            # out = feat_res + msg * gate
            for ct in range(2):
                nc.vector.scalar_tensor_tensor(
                    out=msg_sb[:, ct * O:(ct + 1) * O],
                    in0=msg_sb[:, ct * O:(ct + 1) * O],
                    scalar=gate[:, ct:ct + 1],
                    in1=sb_featres[:, ct * O:(ct + 1) * O],
                    op0=ALU.mult, op1=ALU.add)
                nc.sync.dma_start(out_d[ct * 128:(ct + 1) * 128, :],
                                  msg_sb[:, ct * O:(ct + 1) * O])

    nc.compile()
    return nc


def _prep_inputs(inputs):
    bf = ml_dtypes.bfloat16
    f = lambda x: np.ascontiguousarray(np.asarray(x, dtype=np.float32))
    feat = f(inputs["feat"])
    Wq, Wk, Wv = f(inputs["Wq"]), f(inputs["Wk"]), f(inputs["Wv"])
    bq, bv = f(inputs["bq"]), f(inputs["bv"])
    W1, W2, W3 = f(inputs["W1"]), f(inputs["W2"]), f(inputs["W3"])
    b1, b2, b3 = f(inputs["b1"]), f(inputs["b2"]), f(inputs["b3"])
    g1, be1, m1, v1 = f(inputs["g1"]), f(inputs["be1"]), f(inputs["m1"]), f(inputs["v1"])
    g2, be2, m2, v2 = f(inputs["g2"]), f(inputs["be2"]), f(inputs["m2"]), f(inputs["v2"])
    Wse1, Wse2 = f(inputs["Wse1"]), f(inputs["Wse2"])
    bse1, bse2 = f(inputs["bse1"]), f(inputs["bse2"])

    s1 = g1 / np.sqrt(v1 + EPS)
    sh1 = be1 - m1 * s1 + b1 * s1
    W1p = W1 * s1[:, None]
    s2 = g2 / np.sqrt(v2 + EPS)
    sh2 = be2 - m2 * s2 + b2 * s2
    W2p = W2 * s2[:, None]

    w1_4 = np.concatenate(
        [np.ascontiguousarray(W1p[:, 64 * h:64 * h + 64].T) for h in range(4)],
        axis=1)                                            # [64, 4*128]
    wsq = (Wse1 @ W3) / np.float32(N)                      # [16, 128]
    bsqv = Wse1 @ b3 + bse1                                # [16]

    common = {
        "wq_t": np.ascontiguousarray(Wq.T).astype(bf),
        "wk_t": np.ascontiguousarray(Wk.T).astype(bf),
        "wv_t": np.ascontiguousarray(Wv.T).astype(bf),
        "bq2": np.ascontiguousarray(bq.reshape(2, 128).T),
        "w1_4": np.ascontiguousarray(w1_4).astype(bf),
        "w2_t": np.ascontiguousarray(W2p.T).astype(bf),
        "w3_t": np.ascontiguousarray(W3.T).astype(bf),
        "bn1_b": sh1.reshape(128, 1),
        "bn2_b": sh2.reshape(128, 1),
        "b3_2": np.ascontiguousarray(b3.reshape(2, 128).T),
        "wsq_t": np.ascontiguousarray(wsq.T).astype(bf),
        "bsq": bsqv.reshape(16, 1),
        "wse2_t": np.ascontiguousarray(Wse2.T).astype(bf),
        "bse2_2": np.ascontiguousarray((-bse2).reshape(2, 128).T),
    }

    in_maps = []
    for core in range(8):
        b, osl = core // 4, core % 4
        o0 = osl * O
        fb = feat[b]
        m = dict(common)
        m["feat_bf"] = fb.astype(bf)
        m["feat_q"] = np.ascontiguousarray(fb[:, o0:o0 + O]).astype(bf)
        m["feat_res"] = np.ascontiguousarray(fb[:, o0:o0 + O])
        fbv = fb[:, o0:o0 + O] - bv[:, None]
        m["feat_bv4"] = np.ascontiguousarray(
            np.concatenate([fbv[64 * h:64 * h + 64, :] for h in range(4)],
                           axis=1)).astype(bf)
        in_maps.append(m)
    return in_maps


def kernel(**inputs) -> np.ndarray:
    if "nc" not in _CACHE:
        _CACHE["nc"] = _build()
    nc = _CACHE["nc"]
    in_maps = _prep_inputs(inputs)
    res = run_bass_kernel_spmd(nc, in_maps, core_ids=list(range(8)))
    out = np.zeros((BS, C, N), dtype=np.float32)
    for core in range(8):
        b, osl = core // 4, core % 4
        out[b, :, osl * O:(osl + 1) * O] = res.results[core]["out"]
    return out


if __name__ == "__main__":
    import sys
    sys.path.insert(0, "/root/problem")
    from reference import setup_inputs, reference
    inp = {k: np.asarray(v) for k, v in setup_inputs().items()}
    ref = np.asarray(reference(**inp))
    got = kernel(**inp)
    err = np.abs(got - ref)
    print("absmax err:", err.max(), "ref absmax:", np.abs(ref).max())
    print("Relative error:", err.max() / np.abs(ref).max())
